# revision 29
# baseline (speedup 1.0000x reference)
"""Trainium2 Bass kernel for fused LN + QKV + partial-RoPE attention + out-proj.

Sharding: 8 cores = 4 batches x 2 head-groups (4 heads each).
Core c: batch = c % 4, heads = [4*(c//4) .. 4*(c//4)+3].
Each core returns a partial y^T [DIM, N]; host sums the two head-group
partials per batch and adds b_out.

Device design (per core):
  - LayerNorm in token-major tiles [128n, 128d]; stats batched [128, 16].
  - xn transposed via PE into xnT [DIM, N] (bf16).
  - Per head: qT/rotqT/kT/rotkT = W @ xnT (rotation folded into host
    precomputed weights); RoPE = q*cos + rot*sin on DVE; v computed
    token-major directly.
  - Attention with TRANSPOSED scores: scoresT[j, q] = k_jb^T-stationary
    matmuls, so probs come out with j on partitions and feed the AV matmul
    with no transposes. exp on ACT (scale folds 1/sqrt(d)); row sums via
    all-ones stationary matmul (output is the broadcast R); 1/R via
    ACT exp(-ln(R)) (same table set as Exp); normalize folded into the
    PSUM->SBUF copy of the AV output on DVE.
"""

import numpy as np
import ml_dtypes
from contextlib import ExitStack

import concourse.bass as bass
import concourse.tile as tile
from concourse import bacc
from concourse import mybir
from concourse.bass import ts
from concourse.bass_utils import run_bass_kernel_spmd

B, N, DIM = 4, 2048, 128
HEADS, HEAD = 8, 128
INNER = HEADS * HEAD
HPC = 4            # heads per core
NT = N // 128      # 16 token tiles
EPS = 1e-5
SCALE = HEAD ** -0.5

F32 = mybir.dt.float32
BF16 = mybir.dt.bfloat16
AF = mybir.ActivationFunctionType
ALU = mybir.AluOpType
AX = mybir.AxisListType

BF16_NP = ml_dtypes.bfloat16

_CACHE = {}


def _build_nc():
    nc = bacc.Bacc()
    x_d = nc.declare_dram_parameter("x", [N, DIM], F32, isOutput=False)
    wqkv_d = nc.declare_dram_parameter("wqkv", [128, HPC * 5 * 128], BF16, isOutput=False)
    wo_d = nc.declare_dram_parameter("wo", [128, HPC * 128], BF16, isOutput=False)
    cos_d = nc.declare_dram_parameter("cost", [128, N], F32, isOutput=False)
    sin_d = nc.declare_dram_parameter("sint", [128, N], F32, isOutput=False)
    ident_d = nc.declare_dram_parameter("ident", [128, 128], BF16, isOutput=False)
    ones_d = nc.declare_dram_parameter("ones", [128, 128], BF16, isOutput=False)
    yt_d = nc.declare_dram_parameter("yt", [128, N], F32, isOutput=True)

    with ExitStack() as ctx:
        tc = ctx.enter_context(tile.TileContext(nc))
        const = ctx.enter_context(tc.tile_pool(name="const", bufs=1))
        sb = ctx.enter_context(tc.tile_pool(name="sb", bufs=1))
        rope_p = ctx.enter_context(tc.tile_pool(name="rope", bufs=2))
        qk_p = ctx.enter_context(tc.tile_pool(name="qk", bufs=HPC))
        exp_p = ctx.enter_context(tc.tile_pool(name="exps", bufs=6))
        on_p = ctx.enter_context(tc.tile_pool(name="onorm", bufs=4 * HPC))
        rv_p = ctx.enter_context(tc.tile_pool(name="rv", bufs=4 * HPC))
        ps_sc = ctx.enter_context(tc.tile_pool(name="ps_sc", bufs=4, space="PSUM"))
        ps_av = ctx.enter_context(tc.tile_pool(name="ps_av", bufs=2, space="PSUM"))
        ps_r = ctx.enter_context(tc.tile_pool(name="ps_r", bufs=2, space="PSUM"))

        # ---------------- constants ----------------
        cos_t = const.tile([128, N], F32, tag="cos")
        nc.sync.dma_start(out=cos_t, in_=cos_d[:, :])
        sin_t = const.tile([128, N], F32, tag="sin")
        nc.sync.dma_start(out=sin_t, in_=sin_d[:, :])
        wqkv_t = const.tile([128, HPC * 5 * 128], BF16, tag="wqkv")
        nc.sync.dma_start(out=wqkv_t, in_=wqkv_d[:, :])
        wo_t = const.tile([128, HPC * 128], BF16, tag="wo")
        nc.sync.dma_start(out=wo_t, in_=wo_d[:, :])
        ident_t = const.tile([128, 128], BF16, tag="ident")
        nc.sync.dma_start(out=ident_t, in_=ident_d[:, :])
        ones_t = const.tile([128, 128], BF16, tag="ones")
        nc.sync.dma_start(out=ones_t, in_=ones_d[:, :])

        def W(h, i):
            return wqkv_t[:, ts(h * 5 + i, 128)]

        # ---------------- LayerNorm ----------------
        xt_p = ctx.enter_context(tc.tile_pool(name="xt", bufs=NT))
        xts = []
        for t in range(NT):
            xt = xt_p.tile([128, 128], F32, tag="xt")
            nc.sync.dma_start(out=xt, in_=x_d[t * 128:(t + 1) * 128, :])
            xts.append(xt)

        st_sum = const.tile([128, NT], F32, tag="st_sum")
        st_sq = const.tile([128, NT], F32, tag="st_sq")
        sq_p = ctx.enter_context(tc.tile_pool(name="sq", bufs=3))
        for t in range(NT):
            nc.vector.tensor_reduce(
                out=st_sum[:, t:t + 1], in_=xts[t], axis=AX.X, op=ALU.add)
            sq = sq_p.tile([128, 128], F32, tag="sq")
            nc.vector.tensor_mul(sq, xts[t], xts[t])
            nc.vector.tensor_reduce(
                out=st_sq[:, t:t + 1], in_=sq, axis=AX.X, op=ALU.add)

        mean = const.tile([128, NT], F32, tag="mean")
        nc.vector.tensor_scalar_mul(mean, st_sum, 1.0 / DIM)
        msq = const.tile([128, NT], F32, tag="msq")
        nc.scalar.activation(out=msq, in_=mean, func=AF.Square)
        var = const.tile([128, NT], F32, tag="var")
        nc.vector.scalar_tensor_tensor(
            out=var, in0=st_sq, scalar=1.0 / DIM, in1=msq,
            op0=ALU.mult, op1=ALU.subtract)
        epsb = const.tile([128, 1], F32, tag="epsb")
        nc.vector.memset(epsb, EPS)
        lnv = const.tile([128, NT], F32, tag="lnv")
        nc.scalar.activation(out=lnv, in_=var, func=AF.Ln, bias=epsb)
        istd = const.tile([128, NT], F32, tag="istd")
        nc.scalar.activation(out=istd, in_=lnv, func=AF.Exp, scale=-0.5)
        nbias = const.tile([128, NT], F32, tag="nbias")
        nc.vector.scalar_tensor_tensor(
            out=nbias, in0=mean, scalar=-1.0, in1=istd,
            op0=ALU.mult, op1=ALU.mult)

        xn = const.tile([128, N], BF16, tag="xn")
        for t in range(NT):
            nc.vector.tensor_scalar(
                xn[:, ts(t, 128)], xts[t], mean[:, t:t + 1], istd[:, t:t + 1],
                ALU.subtract, ALU.mult)

        # transpose xn -> xnT [DIM, N]
        xnT = const.tile([128, N], BF16, tag="xnT")
        for qq in range(4):
            xnT_ps = ps_sc.tile([128, 512], BF16, tag="sc")
            for t in range(4):
                nc.tensor.transpose(
                    out=xnT_ps[:, ts(t, 128)], in_=xn[:, ts(qq * 4 + t, 128)],
                    identity=ident_t)
            nc.vector.tensor_copy(xnT[:, ts(qq, 512)], xnT_ps)

        # ---------------- per-head attention ----------------
        # All PSUM traffic in [128, 1024] half-q granularity so that
        # double-buffered scores + AV-accum + R-accum fit in 8 banks.
        onorm = {}
        qhs, khs, vhs = {}, {}, {}
        for h in range(HPC):
            # --- Q/K projections + rope (per half to stay in sc tiles)
            qh = qk_p.tile([128, N], BF16, tag="qrope")
            kh = qk_p.tile([128, N], BF16, tag="krope")
            for dst, wi, wri in ((qh, 0, 1), (kh, 2, 3)):
                for qq in range(4):
                    p_ps = ps_sc.tile([128, 512], F32, tag="sc")
                    nc.tensor.matmul(out=p_ps, lhsT=W(h, wi),
                                     rhs=xnT[:, ts(qq, 512)],
                                     start=True, stop=True)
                    pr_ps = ps_sc.tile([128, 512], F32, tag="sc")
                    nc.tensor.matmul(out=pr_ps, lhsT=W(h, wri),
                                     rhs=xnT[:, ts(qq, 512)],
                                     start=True, stop=True)
                    t1 = rope_p.tile([128, 512], F32, tag="rope1")
                    nc.vector.tensor_mul(t1, p_ps, cos_t[:, ts(qq, 512)])
                    t2 = rope_p.tile([128, 512], F32, tag="rope2")
                    nc.vector.tensor_mul(t2, pr_ps, sin_t[:, ts(qq, 512)])
                    nc.vector.tensor_add(dst[:, ts(qq, 512)], t1, t2)

            # --- V token-major (two sc tiles of 8 chunks each)
            vh = qk_p.tile([128, N], BF16, tag="vsb")
            for qq in range(4):
                v_ps = ps_sc.tile([128, 512], F32, tag="sc")
                for c in range(4):
                    nc.tensor.matmul(out=v_ps[:, ts(c, 128)],
                                     lhsT=xnT[:, ts(qq * 4 + c, 128)],
                                     rhs=W(h, 4), start=True, stop=True)
                nc.vector.tensor_copy(vh[:, ts(qq, 512)], v_ps)
            qhs[h], khs[h], vhs[h] = qh, kh, vh

        ounn = {}
        rinvs = {}
        for h in range(HPC):
            qh, kh, vh = qhs[h], khs[h], vhs[h]
            # --- attention, q processed in 512-wide blocks (1-bank PSUM tiles)
            for qb in range(4):
                qslice = qh[:, ts(qb, 512)]
                o_ps = ps_av.tile([128, 512], F32, tag="av")
                R_ps = ps_r.tile([128, 512], F32, tag="r")
                for jb in range(NT):
                    s_ps = ps_sc.tile([128, 512], F32, tag="sc")
                    nc.tensor.matmul(out=s_ps, lhsT=kh[:, ts(jb, 128)],
                                     rhs=qslice, start=True, stop=True)
                    e = exp_p.tile([128, 512], BF16, tag="expT")
                    nc.scalar.activation(out=e, in_=s_ps, func=AF.Exp, scale=SCALE)
                    nc.tensor.matmul(out=R_ps, lhsT=ones_t, rhs=e,
                                     start=(jb == 0), stop=(jb == NT - 1),
                                     skip_group_check=True)
                    nc.tensor.matmul(out=o_ps, lhsT=vh[:, ts(jb, 128)], rhs=e,
                                     start=(jb == 0), stop=(jb == NT - 1),
                                     skip_group_check=True)
                # evacuate accumulators off the critical path; normalize later
                rinv = rv_p.tile([128, 512], F32, tag="rinv")
                nc.vector.reciprocal_approx_fast(out=rinv, in_=R_ps)
                rinvs[(h, qb)] = rinv
                ou = on_p.tile([128, 512], F32, tag="onorm")
                nc.vector.tensor_copy(ou, o_ps)
                ounn[(h, qb)] = ou

        # ---------------- normalize + output projection ----------------
        y_sb = sb.tile([128, N], F32, tag="ysb")
        on2_p = ctx.enter_context(tc.tile_pool(name="onorm2", bufs=4 * HPC))
        for h in range(HPC):
            for qb in range(4):
                onb = on2_p.tile([128, 512], BF16, tag="onormb")
                nc.vector.tensor_mul(onb, ounn[(h, qb)], rinvs[(h, qb)])
                ounn[(h, qb)] = onb
        for qb in range(4):
            y_ps = ps_av.tile([128, 512], F32, tag="av")
            for h in range(HPC):
                nc.tensor.matmul(out=y_ps, lhsT=wo_t[:, ts(h, 128)],
                                 rhs=ounn[(h, qb)],
                                 start=(h == 0), stop=(h == HPC - 1),
                                 skip_group_check=True)
            nc.vector.tensor_copy(y_sb[:, ts(qb, 512)], y_ps)
        nc.sync.dma_start(out=yt_d[:, :], in_=y_sb)


    nc.finalize()
    return nc



def _make_runner(nc, n_cores=8):
    """Cached jitted multi-core executor (mirrors bass2jax.run_bass_via_pjrt,
    minus output-donation so it can be called repeatedly for timing)."""
    import jax
    import jax.numpy as jnp
    from jax.sharding import Mesh, PartitionSpec
    from jax.experimental.shard_map import shard_map
    from concourse import bass2jax, mybir as mb
    bass2jax.install_neuronx_cc_hook()

    partition_name = nc.partition_id_tensor.name if nc.partition_id_tensor else None
    in_names, out_names, out_avals, zero_outs = [], [], [], []
    for alloc in nc.m.functions[0].allocations:
        if not isinstance(alloc, mb.MemoryLocationSet):
            continue
        name = alloc.memorylocations[0].name
        if alloc.kind == "ExternalInput":
            if name != partition_name:
                in_names.append(name)
        elif alloc.kind == "ExternalOutput":
            out_names.append(name)
            shape = tuple(alloc.tensor_shape)
            dtype = mb.dt.np(alloc.dtype)
            out_avals.append(jax.core.ShapedArray(shape, dtype))
            zero_outs.append(np.zeros(shape, dtype))
    n_params = len(in_names)
    all_in_names = list(in_names) + list(out_names)
    if partition_name is not None:
        all_in_names.append(partition_name)

    def _body(*args):
        operands = list(args)
        if partition_name is not None:
            operands.append(bass2jax.partition_id_tensor())
        outs = bass2jax._bass_exec_p.bind(
            *operands,
            out_avals=tuple(out_avals),
            in_names=tuple(all_in_names),
            out_names=tuple(out_names),
            lowering_input_output_aliases=(),
            sim_require_finite=True,
            sim_require_nnan=True,
            nc=nc,
        )
        return tuple(outs)

    devices = jax.devices()[:n_cores]
    mesh = Mesh(np.asarray(devices), ("core",))
    in_specs = (PartitionSpec("core"),) * (n_params + len(out_names))
    out_specs = (PartitionSpec("core"),) * len(out_names)
    donate = tuple(range(n_params, n_params + len(out_names)))
    sharded = jax.jit(shard_map(_body, mesh=mesh, in_specs=in_specs,
                                out_specs=out_specs, check_rep=False),
                      donate_argnums=donate, keep_unused=True)

    def run(in_maps):
        concat_in = [np.concatenate([np.asarray(in_maps[c][k]) for c in range(n_cores)], axis=0)
                     for k in in_names]
        concat_zero = [np.concatenate([z] * n_cores, axis=0) for z in zero_outs]
        outs = sharded(*concat_in, *concat_zero)
        outs = [np.asarray(o) for o in outs]
        res = []
        for c in range(n_cores):
            d = {}
            for i, name in enumerate(out_names):
                per = outs[i].shape[0] // n_cores
                d[name] = outs[i][c * per:(c + 1) * per]
            res.append(d)
        return res, sharded, (in_names, zero_outs)

    return run


def _rope_tables():
    """cos/sin tables in [d, n] layout; token N-1 unrotated; sin sign-folded."""
    inv_freq = 1.0 / (10000.0 ** (np.arange(0, HEAD, 2, dtype=np.float64) / HEAD))
    pos = np.arange(N, dtype=np.float64)
    ang = pos[None, :] * np.repeat(inv_freq, 2)[:, None]        # [d, n]
    cos_t = np.cos(ang)
    sin_t = np.sin(ang)
    sign = np.where(np.arange(HEAD) % 2 == 0, -1.0, 1.0)[:, None]
    sin_t = sin_t * sign
    cos_t[:, N - 1] = 1.0
    sin_t[:, N - 1] = 0.0
    return cos_t.astype(np.float32), sin_t.astype(np.float32)


def _prep_core_inputs(x, ln_gamma, ln_beta, w_qkv, w_out):
    """Build the 8 per-core input maps (host-side layout/packing)."""
    cos_t, sin_t = _rope_tables()
    ident = np.eye(128, dtype=np.float32)
    ones = np.ones((128, 128), dtype=np.float32)

    swap = np.arange(HEAD) ^ 1                                  # pair swap perm
    in_maps = []
    for c in range(8):
        b = c % 4
        g = c // 4
        wq_blocks = []
        for i in range(HPC):
            h = g * HPC + i
            Wq = w_qkv[h * HEAD:(h + 1) * HEAD, :] * ln_gamma[None, :]
            Wk = w_qkv[INNER + h * HEAD:INNER + (h + 1) * HEAD, :] * ln_gamma[None, :]
            Wv = w_qkv[2 * INNER + h * HEAD:2 * INNER + (h + 1) * HEAD, :] * ln_gamma[None, :]
            wq_blocks += [Wq.T, Wq[swap, :].T, Wk.T, Wk[swap, :].T, Wv.T]
        wqkv_packed = np.concatenate(wq_blocks, axis=1)          # [128, HPC*5*128]
        wo_packed = np.concatenate(
            [w_out[:, (g * HPC + i) * HEAD:(g * HPC + i + 1) * HEAD].T
             for i in range(HPC)], axis=1)                       # [d, HPC*128] -> lhsT per head
        in_maps.append({
            "x": np.ascontiguousarray(x[b], dtype=np.float32),
            "wqkv": wqkv_packed.astype(BF16_NP),
            "wo": wo_packed.astype(BF16_NP),
            "cost": cos_t,
            "sint": sin_t,
            "ident": ident.astype(BF16_NP),
            "ones": ones.astype(BF16_NP),
        })
    return in_maps


def kernel(x, ln_gamma, ln_beta, w_qkv, w_out, b_out):
    x = np.asarray(x, dtype=np.float32)
    ln_gamma = np.asarray(ln_gamma, dtype=np.float32)
    ln_beta = np.asarray(ln_beta, dtype=np.float32)
    w_qkv = np.asarray(w_qkv, dtype=np.float32)
    w_out = np.asarray(w_out, dtype=np.float32)
    b_out = np.asarray(b_out, dtype=np.float32)
    assert np.allclose(ln_beta, 0.0), "beta folding not implemented"

    if "nc" not in _CACHE:
        _CACHE["nc"] = _build_nc()
    nc = _CACHE["nc"]

    in_maps = _prep_core_inputs(x, ln_gamma, ln_beta, w_qkv, w_out)
    _CACHE["last_in_maps"] = in_maps
    res = run_bass_kernel_spmd(nc, in_maps, list(range(8)))
    results = res.results

    out = np.empty((B, N, DIM), dtype=np.float32)
    for b in range(B):
        y0 = np.asarray(results[b]["yt"], dtype=np.float32)
        y1 = np.asarray(results[b + 4]["yt"], dtype=np.float32)
        out[b] = (y0 + y1).T + b_out[None, :]
    return out
